# revision 7
# baseline (speedup 1.0000x reference)
"""Trainium2 Bass kernel: collaborative-filtering score (segment_reduce problem).

Math (per batch element b):
    ubf[u]    = masked mean over nonzero entries of rating_mtx[u, :]
    score[b]  = sum_u  S[user_b, u] * (R[u, item_b] - ubf[u])
    out[b]    = 5 * sigmoid(score[b] + user_bias[user_b] + item_bias[item_b] + gb)

Distribution: BATCH-sharded. Core k owns batch slice [k*1024, (k+1)*1024) and
gathers FULL 8192-wide fp16 rows of the (replicated) augmented similarity
table (by user idx) and of the transposed rating table (by item idx). This is
8x fewer gathered rows than user-sharding (GPSIMD descriptor-gen cost drops
8x) and needs NO score AllReduce: each core applies the sigmoid to its own
1024 scores and writes its own output shard; the host concatenates.

ubf is computed u-sharded (hint: "masked mean shards along n_users rows with
an all-gather of user_bias_fixed"): core k streams its fp8 natural-layout
slice (R[k*1024:(k+1)*1024, :] - 2.5, exact in e4m3), reduces along the free
axis (sum on ACT via accum_out, count on DVE via accum_out), and the [128, 8]
result is ALREADY in the per-partition-scalar layout the products need. A
4 KB AllGather distributes it; a strided DMA lands the full [128, 64]
ubf-column tile with no on-chip transpose.

Main loop (8 chunks of 128 batch elems): transposed gathers land [u'-part,
65 f-groups, 128 batch] fp16 tiles; DVE computes (A + (2.5-ubf)) then *G
(tables store R-2.5, so the 2.5 cancels; centering shrinks fp16 rounding),
plus one tree level folding 65 groups -> 32; the PE (otherwise idle) does the
partition reduction as 32 accumulating ones-matmuls into [1,128] f32 PSUM.
Biases ride as hi/lo fp16 augmented columns (group 64) folded into the same
dot product. ACT applies sigmoid*5 per chunk; scores stream out as computed.

Tables are fp16 (NOT bf16): S values are ~N(0, 0.011) so fp16's 10-bit
mantissa cuts quantization noise ~8x vs bf16; centered ratings and the
hi/lo-split biases are exact.
"""

import sys
from dataclasses import dataclass

import numpy as np

if "/opt/trn_rl_repo" not in sys.path:
    sys.path.insert(0, "/opt/trn_rl_repo")


@dataclass(frozen=True)
class Cfg:
    n_users: int = 8192
    n_items: int = 4096
    batch: int = 8192
    n_cores: int = 8
    chunk: int = 128  # batch elems per gather chunk
    act_add_chunks: int = 0  # chunks whose ubf-add runs per-group on ACT

    @property
    def bl(self) -> int:  # batch elems per core
        return self.batch // self.n_cores

    @property
    def w(self) -> int:
        # gather-row width: n_users data cols + 4 bias cols, padded to a
        # multiple of 128 elems (fp16 -> 256B rows, dma_gather constraint)
        return ((self.n_users + 4 + 127) // 128) * 128


def build_program(cfg: Cfg):
    from concourse import bacc, mybir, tile

    f32 = mybir.dt.float32
    f16 = mybir.dt.float16
    f8 = mybir.dt.float8e4
    i16 = mybir.dt.int16
    Alu = mybir.AluOpType
    Act = mybir.ActivationFunctionType

    U, I, B, W = cfg.n_users, cfg.n_items, cfg.batch, cfg.w
    BL, CH = cfg.bl, cfg.chunk
    F = W // 128  # f-groups per gather row (65)
    FD = F - 1  # data groups (64)
    NCH = BL // CH  # main-loop chunks (8)
    ICN = CH // 16  # idx-table cols per chunk
    RT = BL // 128  # rnat row-tiles (8)
    IDXC = BL // 16
    groups = [list(range(cfg.n_cores))]

    nc = bacc.Bacc(
        None, target_bir_lowering=False, debug=False, num_swdge_queues=2
    )

    sim_t = nc.dram_tensor("sim_aug", [U, W], f16, kind="ExternalInput")
    rtt_t = nc.dram_tensor("ratt_aug", [I, W], f16, kind="ExternalInput")
    rnat_t = nc.dram_tensor("rnat", [BL, I], f8, kind="ExternalInput")
    uidx_t = nc.dram_tensor("uidx", [128, IDXC], i16, kind="ExternalInput")
    iidx_t = nc.dram_tensor("iidx", [128, IDXC], i16, kind="ExternalInput")
    out_t = nc.dram_tensor("out", [BL], f32, kind="ExternalOutput")

    with tile.TileContext(nc) as tc:
        with (
            tc.tile_pool(name="static", bufs=1) as st,
            tc.tile_pool(name="rstream", bufs=3) as rpool,
            tc.tile_pool(name="scr", bufs=2) as scr,
            tc.tile_pool(name="gpool", bufs=3) as gpool,
            tc.tile_pool(name="apool", bufs=3) as apool,
            tc.tile_pool(name="prodp", bufs=2) as ppool,
            tc.tile_pool(name="finp", bufs=2) as fpool,
            tc.tile_pool(name="psB", bufs=2, space="PSUM") as psB,
            tc.tile_pool(name="dram", bufs=1, space="DRAM") as dram,
        ):
            # ---- static setup ----
            ones_w = st.tile([128, 1], f16)
            nc.gpsimd.memset(ones_w[:], 1.0)
            two_b = st.tile([128, 1], f32)
            nc.gpsimd.memset(two_b[:], 2.0)
            uidx_sb = st.tile([128, IDXC], i16)
            nc.sync.dma_start(out=uidx_sb[:], in_=uidx_t[:])
            iidx_sb = st.tile([128, IDXC], i16)
            nc.sync.dma_start(out=iidx_sb[:], in_=iidx_t[:])

            # ---- ubf local pass: free-axis reduce of the fp8 natural slice
            # sum on ACT (Copy+accum), count on DVE (not_equal+accum); both
            # land [128, RT] f32: partition p, col j <-> local user j*128+p.
            sum_acc = st.tile([128, RT], f32)
            cnt_acc = st.tile([128, RT], f32)
            for j in range(RT):
                rt = rpool.tile([128, I], f8, name="rt")
                nc.sync.dma_start(
                    out=rt[:], in_=rnat_t[j * 128 : (j + 1) * 128, :]
                )
                s1 = scr.tile([128, I], f8, name="s1")
                nc.scalar.activation(
                    out=s1[:], in_=rt[:], func=Act.Copy,
                    accum_out=sum_acc[:, j : j + 1],
                )
                # count via ACT accum of Sign(x+2.0): x==-2.5 (rating 0) ->
                # -1, else >= +1.5 -> +1, so accum = 2*cnt - I. (DVE
                # tensor_scalar accum_out returns 0 on HW; ACT accum works.)
                s2 = scr.tile([128, I], f8, name="s2")
                nc.scalar.activation(
                    out=s2[:], in_=rt[:], func=Act.Sign, bias=two_b[:],
                    accum_out=cnt_acc[:, j : j + 1],
                )
            # ubf math (values are R-2.5; sum_R = sum_acc + 2.5*I):
            #   ubf  = sum_R / max(cnt, 1)      (0 when cnt==0)
            #   ubfn = -(ubf - 2.5) = 2.5 - ubf (add-ready for the products;
            #   cnt==0 -> sum_R==0 -> ubf==0 -> ubfn==2.5 which matches the
            #   reference's adjusted = R - 0 in centered form)
            cntm = st.tile([128, RT], f32)
            nc.vector.tensor_scalar(
                out=cntm[:], in0=cnt_acc[:], scalar1=0.5, scalar2=float(I) / 2,
                op0=Alu.mult, op1=Alu.add,
            )
            nc.vector.tensor_scalar(
                out=cntm[:], in0=cntm[:], scalar1=1.0, scalar2=None,
                op0=Alu.max,
            )
            nc.vector.reciprocal(out=cntm[:], in_=cntm[:])
            ubfn = st.tile([128, RT], f32)
            nc.vector.tensor_scalar(
                out=ubfn[:], in0=sum_acc[:], scalar1=2.5 * I, scalar2=None,
                op0=Alu.add,
            )
            nc.vector.tensor_tensor(
                out=ubfn[:], in0=ubfn[:], in1=cntm[:], op=Alu.mult
            )
            nc.vector.tensor_scalar(
                out=ubfn[:], in0=ubfn[:], scalar1=-1.0, scalar2=2.5,
                op0=Alu.mult, op1=Alu.add,
            )

            # ---- AllGather ubfn across cores ----
            # DRAM layout per core: element p*RT+j (p-major), so rank r's
            # block concatenated at offset r*BL keeps a uniform formula:
            # rd[r*BL + p*RT + j] = ubfn(user r*BL + j*128 + p).
            pd = dram.tile([1, BL], f32, name="ubf_part")
            nc.sync.dma_start(
                out=pd[:].rearrange("o (p j) -> (o p) j", p=128), in_=ubfn[:]
            )
            rd = dram.tile([1, U], f32, name="ubf_all", addr_space="Shared")
            nc.gpsimd.collective_compute(
                "AllGather", Alu.bypass, replica_groups=groups,
                ins=[pd.opt()], outs=[rd.opt()],
            )
            # land as [128, F] fp16: partition p, col f = ubfn(user f*128+p)
            # (f = r*RT+j since r*BL = r*RT*128); group FD (aug cols) = 0.
            ubf_g = st.tile([128, cfg.n_cores, RT], f32)
            nc.sync.dma_start(
                out=ubf_g[:],
                in_=rd[:].rearrange(
                    "o (r p j) -> (o p) r j", r=cfg.n_cores, p=128
                ),
            )
            ubf_colT = st.tile([128, F], f16)
            nc.gpsimd.memset(ubf_colT[:], 0.0)
            nc.vector.tensor_copy(
                out=ubf_colT[:, :FD],
                in_=ubf_g[:].rearrange("p r j -> p (r j)"),
            )
            # physical broadcast [128, F, CH] so the per-chunk add is ONE op
            ubf_bc = st.tile([128, F, CH], f16)
            nc.vector.tensor_copy(
                out=ubf_bc[:], in_=ubf_colT[:, :, None].broadcast_to([128, F, CH])
            )

            # ---- main loop ----
            out_v = out_t[:].rearrange("(k c) -> k c", k=NCH)
            for k in range(NCH):
                gk = gpool.tile([128, F, CH], f16, name="gk")
                nc.gpsimd.dma_gather(
                    out_ap=gk[:], in_ap=sim_t[:],
                    idxs_ap=uidx_sb[:, k * ICN : (k + 1) * ICN],
                    num_idxs=CH, num_idxs_reg=CH, elem_size=W,
                    transpose=True, queue_num=0,
                )
                ak = apool.tile([128, F, CH], f16, name="ak")
                nc.gpsimd.dma_gather(
                    out_ap=ak[:], in_ap=rtt_t[:],
                    idxs_ap=iidx_sb[:, k * ICN : (k + 1) * ICN],
                    num_idxs=CH, num_idxs_reg=CH, elem_size=W,
                    transpose=True, queue_num=1,
                )
                # ubf add: in-place ak += ubf_bc (DVE, or per-group on ACT)
                if k < cfg.act_add_chunks:
                    for f in range(F):
                        nc.scalar.add(
                            out=ak[:, f, :], in_=ak[:, f, :],
                            add=ubf_colT[:, f : f + 1],
                        )
                else:
                    nc.vector.tensor_tensor(
                        out=ak[:], in0=ak[:], in1=ubf_bc[:], op=Alu.add
                    )
                # products + one tree level (65 -> 32 groups)
                p1 = ppool.tile([128, F, CH], f16, name="p1")
                nc.vector.tensor_tensor(
                    out=p1[:], in0=ak[:], in1=gk[:], op=Alu.mult
                )
                nc.vector.tensor_tensor(
                    out=p1[:, 0 : FD // 2, :], in0=p1[:, 0 : FD // 2, :],
                    in1=p1[:, FD // 2 : FD, :], op=Alu.add,
                )
                nc.vector.tensor_tensor(
                    out=p1[:, 0, :], in0=p1[:, 0, :], in1=p1[:, FD, :],
                    op=Alu.add,
                )
                # partition-reduce on the PE: 32 accumulating ones-matmuls
                ps = psB.tile([1, CH], f32, name="ps")
                for f in range(FD // 2):
                    nc.tensor.matmul(
                        out=ps[:], lhsT=ones_w[:], rhs=p1[:, f, :],
                        start=(f == 0), stop=(f == FD // 2 - 1),
                    )
                # finish: 5*sigmoid -> out shard slice
                fin = fpool.tile([1, CH], f32, name="fin")
                nc.scalar.activation(out=fin[:], in_=ps[:], func=Act.Sigmoid)
                nc.vector.tensor_scalar_mul(out=fin[:], in0=fin[:], scalar1=5.0)
                nc.sync.dma_start(out=out_v[k : k + 1, :], in_=fin[:])

    nc.compile()
    return nc


def make_in_maps(cfg, user, item, rating_mtx, user_similarity, user_bias, item_bias, global_bias):
    import ml_dtypes

    U, I, B, W, BL = cfg.n_users, cfg.n_items, cfg.batch, cfg.w, cfg.bl
    f16 = np.float16
    f8 = ml_dtypes.float8_e4m3
    u_i = np.asarray(user).astype(np.int64)
    i_i = np.asarray(item).astype(np.int64)
    sim = np.asarray(user_similarity, dtype=np.float32)
    R = np.asarray(rating_mtx, dtype=np.float32)
    ub = np.asarray(user_bias, dtype=np.float32)
    ibg = np.asarray(item_bias, dtype=np.float32) + np.float32(
        np.asarray(global_bias)
    )

    def hilo(x):
        hi = x.astype(f16)
        lo = (x - hi.astype(np.float32)).astype(f16)
        return hi, lo

    ub_hi, ub_lo = hilo(ub)
    ib_hi, ib_lo = hilo(ibg)

    # shared tables (identical on every core)
    sa = np.zeros((U, W), f16)
    sa[:, :U] = sim.astype(f16)
    sa[:, U] = ub_hi
    sa[:, U + 1] = ub_lo
    sa[:, U + 2] = 1.0
    sa[:, U + 3] = 1.0
    ra = np.zeros((I, W), f16)
    ra[:, :U] = (R.T - 2.5).astype(f16)
    ra[:, U] = 1.0
    ra[:, U + 1] = 1.0
    ra[:, U + 2] = ib_hi
    ra[:, U + 3] = ib_lo

    # idx layout: [16, n/16] block (idx i at [i%16, i//16]) tiled 8x down the
    # partition axis -- each GPSIMD Q7 core reads its own 16-partition replica
    def idx_table(v):
        return np.tile(v.astype(np.int16).reshape(-1, 16).T, (8, 1))

    rc = (R - 2.5).astype(f8)
    maps = []
    for k in range(cfg.n_cores):
        lo, hi = k * BL, (k + 1) * BL
        maps.append({
            "sim_aug": sa,
            "ratt_aug": ra,
            "rnat": rc[lo:hi],
            "uidx": idx_table(u_i[lo:hi]),
            "iidx": idx_table(i_i[lo:hi]),
        })
    return maps


_PROGRAM_CACHE = {}


def _get_program(cfg: Cfg):
    if cfg not in _PROGRAM_CACHE:
        _PROGRAM_CACHE[cfg] = build_program(cfg)
    return _PROGRAM_CACHE[cfg]


def assemble_out(cfg, results):
    return np.concatenate(
        [np.asarray(results[k]["out"], dtype=np.float32).reshape(cfg.bl)
         for k in range(cfg.n_cores)]
    )


def kernel(user, item, rating_mtx, user_similarity, user_bias, item_bias, global_bias):
    from concourse import bass_utils

    cfg = Cfg()
    assert np.asarray(rating_mtx).shape == (cfg.n_users, cfg.n_items)
    assert np.asarray(user).shape == (cfg.batch,)
    nc = _get_program(cfg)
    in_maps = make_in_maps(
        cfg, user, item, rating_mtx, user_similarity, user_bias, item_bias, global_bias
    )
    res = bass_utils.run_bass_kernel_spmd(
        nc, in_maps, core_ids=list(range(cfg.n_cores))
    )
    return assemble_out(cfg, res.results)
